# revision 1
# baseline (speedup 1.0000x reference)
"""DeepEMD Trainium2 kernel: batched 49x49 entropic-OT (Sinkhorn) similarity.

Strategy (8 NeuronCores, data-parallel over batch):
- Each core gets 128 batches. Host prepacks, per (chunk j of 128 channels,
  batch b), an augmented matrix A = [Q | P | 1] (128 x 99) in fp16 (10-bit
  mantissa keeps the end-to-end logits error ~2e-4), sequenced in DRAM so
  every load DMA reads one contiguous span.
- PE computes the Gram G_b = A^T A (99x99, fp32 PSUM) with one accumulating
  fp16 matmul per channel chunk (weights widened to 128 cols to engage
  fast-weight-load; junk rows ignored). G contains Q^T P, P^T Q, column
  sums (ones row) and diag blocks -> the similarity map, norms and weight
  vectors are all cheap fixups of G.
- A per-batch SBUF->SBUF DMA flattens G_b into row b of a [128, 99*99]
  tile: everything after that runs batch-on-partitions, full 128-lane DVE.
- Sinkhorn runs in the *linear* domain (K = exp((sim-1)/eps + 16)) with
  Gauss-Seidel updates us = r/(K vs), vs = c/(K^T us). The reference's 100
  log-domain iterations are converged ~1e-12 by 20; ITERS linear f32
  iterations reach ~2e-5 at 8.
- logits[b] = T * sum(flow * sim) = T * us^T ((K.sim) vs).
- One dma_start runs on a single SDMA engine (~27 GB/s), so loads are split
  into sub-DMAs across both HWDGE queues and flattens are spread across
  gpsimd/sync/scalar issuers to keep many engines streaming.
"""

import os
import sys

import numpy as np

sys.path.insert(0, "/opt/trn_rl_repo")

import concourse.bass as bass
import concourse.bacc as bacc
import concourse.mybir as mybir
from concourse import tile
from concourse.bass_utils import run_bass_kernel_spmd

import ml_dtypes

B_FULL, C, HW = 1024, 512, 49
NCORE = 8
BS = B_FULL // NCORE  # 128 batches per core
NCH = C // 128  # 4 chunks of 128 channels (PE contraction dim)
AC = 2 * HW + 1  # 99 augmented columns [Q | P | 1]
GRP = 16  # batches per DMA group
NGRP = BS // GRP
ITERS = 6
EPS_S = 0.05
TEMP = 12.5 / HW
EXP_BIAS = -4.0  # exp((sim-1)/eps) * e^16 rescale; cancels in us*K*vs

f32 = mybir.dt.float32
f16 = mybir.dt.float16
Alu = mybir.AluOpType
Act = mybir.ActivationFunctionType
AxX = mybir.AxisListType.X


def build_nc(debug=False):
    nc = bacc.Bacc(None, target_bir_lowering=False, debug=debug)
    JW = GRP * AC  # cols per chunk-slab in stage
    aug = nc.declare_dram_parameter(
        "aug", [NGRP, 128, NCH * JW], f16, isOutput=False
    )
    outp = nc.declare_dram_parameter("out", [BS, 1], f32, isOutput=True)

    FW = AC * AC  # 9801 flat row width

    with tile.TileContext(nc) as tc:
        with (
            tc.tile_pool(name="big", bufs=1) as big,
            tc.tile_pool(name="stage", bufs=4) as stg,
            tc.tile_pool(name="gcopy", bufs=8) as gcp,
            tc.tile_pool(name="work", bufs=3) as wrk,
            tc.tile_pool(name="small", bufs=1) as sml,
            tc.tile_pool(name="psum", bufs=8, space="PSUM") as pp,
        ):
            flatG = big.tile([BS, FW], f32, tag="flatG", name="flatG")

            # ---------------- Phase 1: DMA in + Gram + flatten ----------------
            NSPL = 8
            SW = NCH * JW // NSPL
            for g in range(NGRP):
                th = stg.tile([128, NCH * JW], f16, tag="h", name="hg")
                # loads live alone on the SP queue: a flatten on the same
                # FIFO queue would head-of-line block the next group's
                # prefetch behind compute
                for ss in range(NSPL):
                    nc.sync.dma_start(
                        th[:, ss * SW : (ss + 1) * SW],
                        aug[g, :, ss * SW : (ss + 1) * SW],
                    )
                for bb in range(GRP):
                    b = g * GRP + bb
                    ps = pp.tile([128, AC], f32, tag="gram", name="gram")
                    for j in range(NCH):
                        base = j * JW + bb * AC
                        # widen weights to 128 cols (spill into following slab
                        # data -> junk G rows 99..127, never read); the very
                        # last slab position must stay 99 wide
                        wid = AC if (bb == GRP - 1 and j == NCH - 1) else 128
                        nc.tensor.matmul(
                            ps[0:wid, :],
                            th[:, base : base + wid],
                            th[:, base : base + AC],
                            start=(j == 0),
                            stop=(j == NCH - 1),
                        )
                    gs = gcp.tile([AC, AC], f32, tag="gs", name="gs")
                    nc.vector.tensor_copy(gs[:], ps[0:AC, :])
                    # flatten [99, 99] -> one batch-major row; spread issue
                    # cost across gpsimd (SWDGE) + both HWDGE queues
                    dmae = (nc.gpsimd, nc.scalar)[b % 2]
                    dmae.dma_start(flatG[b : b + 1, :], gs[:])

            # ---------------- Phase 1.5: fixup to sim/K/marginals -------------
            G3 = flatG[:].rearrange("p (q c) -> p q c", c=AC)
            qtp = G3[:, 0:HW, HW : 2 * HW]  # [128, 49, 49] raw Q^T P
            ptq = G3[:, HW : 2 * HW, 0:HW]
            sq = flatG[:, (AC - 1) * AC : (AC - 1) * AC + HW]  # 1^T Q
            sp = flatG[:, (AC - 1) * AC + HW : (AC - 1) * AC + 2 * HW]  # 1^T P

            def dview(row0, col0):
                # [128, 49] diagonal view: (row0+m)*99 + col0+m, stride 100
                v = flatG[:, row0 * AC + col0 : row0 * AC + col0 + 1].copy()
                v.ap = mybir.VecI64Pair([list(v.ap[0])] + [[AC + 1, HW]])
                return v

            dq = dview(0, 0)  # diag(QtQ)
            dp = dview(HW, HW)  # diag(PtP)

            def s49(tag):
                return sml.tile([BS, HW], f32, tag=tag, name=tag)

            inq, inp_, t1, t2 = s49("inq"), s49("inp"), s49("t1"), s49("t2")
            aq, ap_ = s49("aq"), s49("ap")
            w1, w2, us, vs = s49("w1"), s49("w2"), s49("us"), s49("vs")
            kv, rkv = s49("kv"), s49("rkv")
            s2 = sml.tile([BS, 1], f32, tag="s2", name="s2")
            ebias = sml.tile([BS, 1], f32, tag="ebias", name="ebias")
            nc.vector.memset(ebias[:], EXP_BIAS)
            # warm the ACT sqrt/exp table sets early (no data deps -> Tile
            # schedules these under the phase-1 DMA shadow while ACT is idle,
            # hiding the ~2.7us-per-set PSEUDO_LOAD_ACT_FUNC_SET cost)
            wrm = sml.tile([BS, 1], f32, tag="wrm", name="wrm")
            nc.vector.memset(wrm[:], 1.0)
            nc.scalar.activation(wrm[:], wrm[:], Act.Sqrt)
            nc.scalar.activation(wrm[:], wrm[:], Act.Exp)
            lg = sml.tile([BS, 1], f32, tag="lg", name="lg")
            lgf = sml.tile([BS, 1], f32, tag="lgf", name="lgf")

            def v3(t):  # [128, 49, 49] view of a [128, 2401] tile
                return t[:].rearrange("p (q c) -> p q c", c=HW)

            def v3t(t):  # transposed view (strides 1, 49)
                return t[:].rearrange("p (q c) -> p c q", c=HW)

            # weight vectors: w = relu(rowsum/49) + 0.001 (unnormalized: the
            # r-normalization cancels in the logits, the c-normalization is a
            # final 1/s2 scale)
            nc.vector.tensor_reduce(w1[:], qtp, axis=AxX, op=Alu.add)
            nc.vector.tensor_reduce(w2[:], ptq, axis=AxX, op=Alu.add)
            for w in (w1, w2):
                nc.vector.tensor_scalar(w[:], w[:], 1.0 / HW, 0.0, Alu.mult, Alu.max)
                nc.vector.tensor_scalar(w[:], w[:], 0.001, None, Alu.add)
            nc.vector.tensor_reduce(s2[:], w2[:], axis=AxX, op=Alu.add)

            for (sx, dx, inv) in ((sq, dq, inq), (sp, dp, inp_)):
                # u = diag - s^2/C ; inv = rsqrt(u) via sqrt LUT+recip+Newton
                nc.vector.tensor_mul(t1[:], sx, sx)
                nc.vector.scalar_tensor_tensor(
                    t2[:], t1[:], -1.0 / C, dx, Alu.mult, Alu.add
                )
                nc.scalar.activation(t1[:], t2[:], Act.Sqrt)
                nc.vector.reciprocal(inv[:], t1[:])
                nc.vector.tensor_mul(t1[:], inv[:], inv[:])
                nc.vector.tensor_mul(t1[:], t1[:], t2[:])
                nc.vector.tensor_scalar(t1[:], t1[:], -0.5, 1.5, Alu.mult, Alu.add)
                nc.vector.tensor_mul(inv[:], inv[:], t1[:])

            rC = 1.0 / np.sqrt(float(C))
            nc.vector.scalar_tensor_tensor(
                aq[:], sq, rC, inq[:], Alu.mult, Alu.mult
            )
            nc.vector.scalar_tensor_tensor(
                ap_[:], sp, rC, inp_[:], Alu.mult, Alu.mult
            )

            simb = big.tile([BS, HW * HW], f32, tag="sim", name="sim")
            Kb = big.tile([BS, HW * HW], f32, tag="K", name="K")
            Ktb = big.tile([BS, HW * HW], f32, tag="Kt", name="Kt")
            b1 = wrk.tile([BS, HW * HW], f32, tag="w", name="b1")
            b3 = wrk.tile([BS, HW * HW], f32, tag="w", name="b3")
            simTb = wrk.tile([BS, HW * HW], f32, tag="w", name="simTb")

            bq = inq[:].unsqueeze(2).broadcast_to([BS, HW, HW])
            bp = inp_[:].unsqueeze(1).broadcast_to([BS, HW, HW])
            nc.vector.tensor_mul(v3(b1), bq, bp)  # B1 = inq x inp
            nc.vector.tensor_mul(v3(simb), qtp, v3(b1))  # B2
            baq = aq[:].unsqueeze(2).broadcast_to([BS, HW, HW])
            bap = ap_[:].unsqueeze(1).broadcast_to([BS, HW, HW])
            nc.vector.tensor_mul(v3(b3), baq, bap)  # B3 = aq x ap
            nc.vector.tensor_sub(v3(simb), v3(simb), v3(b3))  # sim = B2 - B3
            nc.vector.tensor_mul(v3(simTb), ptq, v3t(b1))
            nc.vector.tensor_sub(v3(simTb), v3(simTb), v3t(b3))
            nc.scalar.activation(
                Kb[:], simb[:], Act.Exp, scale=1.0 / EPS_S, bias=ebias[:]
            )
            nc.scalar.activation(
                Ktb[:], simTb[:], Act.Exp, scale=1.0 / EPS_S, bias=ebias[:]
            )

            # ---------------- Phase 2: Sinkhorn (Gauss-Seidel, linear) --------
            tb = wrk.tile([BS, HW * HW], f32, tag="w", name="tb")
            bvs = vs[:].unsqueeze(1).broadcast_to([BS, HW, HW])
            bus = us[:].unsqueeze(1).broadcast_to([BS, HW, HW])
            for it in range(ITERS):
                if it == 0:
                    nc.vector.tensor_reduce(kv[:], v3(Kb), axis=AxX, op=Alu.add)
                else:
                    nc.vector.tensor_mul(v3(tb), v3(Kb), bvs)
                    nc.vector.tensor_reduce(kv[:], v3(tb), axis=AxX, op=Alu.add)
                nc.vector.reciprocal(rkv[:], kv[:])
                nc.vector.tensor_mul(us[:], w1[:], rkv[:])
                nc.vector.tensor_mul(v3(tb), v3(Ktb), bus)
                nc.vector.tensor_reduce(kv[:], v3(tb), axis=AxX, op=Alu.add)
                nc.vector.reciprocal(rkv[:], kv[:])
                nc.vector.tensor_mul(vs[:], w2[:], rkv[:])

            # ---------------- Phase 3: logits ---------------------------------
            nc.vector.tensor_mul(v3(tb), v3(Kb), bvs)
            nc.vector.tensor_mul(tb[:], tb[:], simb[:])
            nc.vector.tensor_reduce(kv[:], v3(tb), axis=AxX, op=Alu.add)
            nc.vector.tensor_mul(kv[:], kv[:], us[:])
            nc.vector.tensor_reduce(lg[:], kv[:], axis=AxX, op=Alu.add)
            nc.vector.reciprocal(rkv[:, 0:1], s2[:])
            nc.vector.scalar_tensor_tensor(
                lgf[:], lg[:], TEMP, rkv[:, 0:1], Alu.mult, Alu.mult
            )  # (lg * T) / s2
            nc.sync.dma_start(outp[:, :], lgf[:])

    nc.compile()
    return nc


_NC = None


def _get_nc():
    global _NC
    if _NC is None:
        _NC = build_nc()
    return _NC


def _prep_in_maps(feature_map1, feature_map2):
    q = np.ascontiguousarray(np.asarray(feature_map1, dtype=np.float32)).reshape(
        B_FULL, C, HW
    )
    p = np.ascontiguousarray(np.asarray(feature_map2, dtype=np.float32)).reshape(
        B_FULL, C, HW
    )
    in_maps = []
    for i in range(NCORE):
        sl = slice(i * BS, (i + 1) * BS)
        a32 = np.empty((NCH, 128, BS, AC), np.float32)
        a32[..., AC - 1] = 1.0
        a32[..., 0:HW] = q[sl].reshape(BS, NCH, 128, HW).transpose(1, 2, 0, 3)
        a32[..., HW : 2 * HW] = p[sl].reshape(BS, NCH, 128, HW).transpose(1, 2, 0, 3)
        aug = a32.astype(np.float16)
        # sequence DRAM as [group, channel-partition, chunk, batch, col] so
        # group loads read contiguous spans
        aug = np.ascontiguousarray(
            aug.reshape(NCH, 128, NGRP, GRP, AC).transpose(2, 1, 0, 3, 4)
        ).reshape(NGRP, 128, NCH * GRP * AC)
        in_maps.append({"aug": aug})
    return in_maps


def run(feature_map1, feature_map2, trace=False):
    in_maps = _prep_in_maps(feature_map1, feature_map2)
    nc = _get_nc()
    res = run_bass_kernel_spmd(nc, in_maps, core_ids=list(range(NCORE)), trace=trace)
    out = np.concatenate(
        [np.asarray(res.results[i]["out"]).reshape(BS) for i in range(NCORE)]
    ).astype(np.float32)
    return out, res


def kernel(feature_map1, feature_map2):
    out, _ = run(feature_map1, feature_map2, trace=False)
    return out



# revision 23
# speedup vs baseline: 1.4104x; 1.4104x over previous
"""DeepEMD Trainium2 kernel: batched 49x49 entropic-OT (Sinkhorn) similarity.

v2 strategy (8 NeuronCores, data-parallel over batch; 128 batches/core):
- Host prepacks per (group of 32 batches) slabs A = [Q | P | 1] (128ch x 99)
  fp16, batch-major within the slab so grams start after partial loads.
- Phase A: 4 big loads (1.6MB sub-DMAs, alone on the sync queue to avoid
  head-of-line blocking), per-batch Gram G_b = A^T A via 4 accumulating
  fp16 matmuls (weights widened to 128 cols to engage fast-weight-load),
  PSUM->SBUF fp16 copies into 16-batch blocks, per-batch flatten DMAs
  (99 descs each) spread across scalar/gpsimd queues (sync joins for the
  last group). w1/w2 row/col-sum reduces run per block on the G-major
  layout during phase A; a PE transpose turns them batch-major.
- Phase B: all big tensors fp16 in a [49, 50]-padded layout so every
  tensor_tensor runs in the DVE 2x packed mode (4B-aligned step-1 rows).
  K = exp(20*sim) directly (exp((sim-1)/eps)*e^20 rescale cancels in the
  final normalization). Kt is built by ACT reading sim transposed. Linear
  Gauss-Seidel Sinkhorn, ITERS=4 with the last vs-update skipped (flow
  mass then = sum(w1), so logits normalize by 1/s1). b3/KS products run
  on gpsimd concurrently with DVE. Final logit = one fused
  tensor_tensor_reduce over the full [49,50] row.
"""

import os
import sys

import numpy as np

sys.path.insert(0, "/opt/trn_rl_repo")

import concourse.bass as bass
import concourse.bacc as bacc
import concourse.mybir as mybir
from concourse import tile
from concourse import masks
from concourse.bass_utils import run_bass_kernel_spmd

B_FULL, C, HW = 1024, 512, 49
NCORE = 8
BS = B_FULL // NCORE  # 128 batches per core
NCH = C // 128  # 4 chunks of 128 channels
AC = 2 * HW + 1  # 99 augmented cols [Q | P | 1]
GRP = 32  # batches per load group
NGRP = BS // GRP
NSPL = 2  # sub-DMAs per group load
FB = 16  # batches per gs block / flatten group
PAD = 50  # padded inner dim (4B-aligned fp16 rows)
FW = HW * PAD  # 2450
ITERS = 4
HALF = True  # skip last vs-update; normalize by s1
EXPB = 1.0 / 0.05  # K = exp(sim/eps) (the e^{-1/eps} factor cancels)
SIMPAD = -600.0
TEMP = 12.5 / HW

f32 = mybir.dt.float32
f16 = mybir.dt.float16
Alu = mybir.AluOpType
Act = mybir.ActivationFunctionType
AxX = mybir.AxisListType.X

SLAB = GRP * NCH * AC  # 12672 cols per group slab


def build_nc(debug=False):
    nc = bacc.Bacc(None, target_bir_lowering=False, debug=debug)
    aug = nc.declare_dram_parameter("aug", [NGRP, 128, SLAB], f16, isOutput=False)
    outp = nc.declare_dram_parameter("out", [BS, 1], f32, isOutput=True)

    with tile.TileContext(nc) as tc:
        with (
            tc.tile_pool(name="big", bufs=1) as big,
            tc.tile_pool(name="stage", bufs=2) as stg,
            tc.tile_pool(name="gblk", bufs=3) as gcp,
            tc.tile_pool(name="small", bufs=1) as sml,
            tc.tile_pool(name="psum", bufs=7, space="PSUM") as pp,
            tc.tile_pool(name="psumw", bufs=1, space="PSUM") as ppw,
        ):
            flatG = big.tile([BS, AC * AC], f16, tag="flatG", name="flatG")

            simb = big.tile([BS, FW], f16, tag="sim", name="sim")
            Kb = big.tile([BS, FW], f16, tag="K", name="K")
            Ktb = big.tile([BS, FW], f16, tag="Kt", name="Kt")
            tb = big.tile([BS, FW], f16, tag="tb", name="tb")
            KS = big.tile([BS, FW], f16, tag="KS", name="KS")
            b3 = big.tile([BS, FW], f16, tag="b3", name="b3")
            # pad prep: sim pads -> exp 0; Kt pad col stays 0
            nc.gpsimd.memset(simb[:], SIMPAD)
            nc.gpsimd.memset(Ktb[:], 0.0)

            def s_t(tag, w=HW, dt=f32):
                return sml.tile([BS, w], dt, tag=tag, name=tag)

            us = s_t("us", PAD, f16)
            vs = s_t("vs", PAD, f16)
            nc.gpsimd.memset(us[:], 0.0)
            nc.gpsimd.memset(vs[:], 0.0)
            # warm ACT sqrt/exp tables early under the load shadow
            wrm = s_t("wrm", 1)
            nc.vector.memset(wrm[:], 1.0)
            nc.scalar.activation(wrm[:], wrm[:], Act.Sqrt)
            nc.scalar.activation(wrm[:], wrm[:], Act.Exp)

            # ---------------- Phase A: load + gram + flatten ----------------
            gs = None
            for g in range(NGRP):
                th = stg.tile([128, SLAB], f16, tag="h", name="hg")
                sw = SLAB // NSPL
                for ss in range(NSPL):
                    nc.sync.dma_start(
                        th[:, ss * sw : (ss + 1) * sw],
                        aug[g, :, ss * sw : (ss + 1) * sw],
                    )
                for b in range(GRP):
                    gb = g * GRP + b
                    blk, bb = gb // FB, gb % FB
                    if bb == 0:
                        gs = gcp.tile([AC, FB * AC], f16, tag="gs", name="gs")
                    ps = pp.tile([128, AC], f32, tag="gram", name="gram")
                    # start/stop must cover one partition range: widen all 4
                    # chunks or none (no widening only for the slab tail)
                    wid = 128 if b * (NCH * AC) + (NCH - 1) * AC + 128 <= SLAB else AC
                    for j in range(NCH):
                        base = b * (NCH * AC) + j * AC
                        nc.tensor.matmul(
                            ps[0:wid, :],
                            th[:, base : base + wid],
                            th[:, base : base + AC],
                            start=(j == 0),
                            stop=(j == NCH - 1),
                        )
                    nc.vector.tensor_copy(gs[:, bb * AC : (bb + 1) * AC], ps[0:AC, :])
                    # flatten G_b -> row gb of flatG (99 descs); loads own the
                    # sync queue until the last group's loads are issued
                    if g < NGRP - 1:
                        dmae = (nc.scalar, nc.gpsimd)[gb % 2]
                    else:
                        dmae = (nc.scalar, nc.gpsimd, nc.sync)[gb % 3]
                    dmae.dma_start(
                        flatG[gb : gb + 1, :], gs[:, bb * AC : (bb + 1) * AC]
                    )


            # ---------------- Phase B: fixups ----------------
            w1r, w2r = s_t("w1r"), s_t("w2r")
            w1f = s_t("w1f", HW, f16)
            w2f = s_t("w2f", HW, f16)
            s1s, s2s = s_t("s1s", 1), s_t("s2s", 1)
            G3e = flatG[:].rearrange("b (q c) -> b q c", c=AC)
            nc.vector.tensor_reduce(
                w1r[:], G3e[:, 0:HW, HW : 2 * HW], axis=AxX, op=Alu.add
            )
            nc.vector.tensor_reduce(
                w2r[:], G3e[:, HW : 2 * HW, 0:HW], axis=AxX, op=Alu.add
            )
            for wr, wf in ((w1r, w1f), (w2r, w2f)):
                nc.vector.tensor_scalar(wr[:], wr[:], 1.0 / HW, 0.0, Alu.mult, Alu.max)
                nc.vector.tensor_scalar(wr[:], wr[:], 0.001, None, Alu.add)
                nc.vector.tensor_copy(wf[:], wr[:])
            if HALF:
                nc.vector.tensor_reduce(s1s[:], w1r[:], axis=AxX, op=Alu.add)
            else:
                nc.vector.tensor_reduce(s2s[:], w2r[:], axis=AxX, op=Alu.add)

            # diag/sum views on flatG (fp16, stride 100 diagonals)
            def dview(row0, col0):
                v = flatG[:, row0 * AC + col0 : row0 * AC + col0 + 1].copy()
                v.ap = mybir.VecI64Pair([list(v.ap[0])] + [[AC + 1, HW]])
                return v

            dq = dview(0, 0)
            dp = dview(HW, HW)
            sq = flatG[:, (AC - 1) * AC : (AC - 1) * AC + HW]
            sp = flatG[:, (AC - 1) * AC + HW : (AC - 1) * AC + 2 * HW]

            inq = s_t("inq", HW, f16)
            inp_ = s_t("inp", HW, f16)
            aq = s_t("aq", HW, f16)
            ap_ = s_t("ap", HW, f16)
            t1, t2, n1 = s_t("t1"), s_t("t2"), s_t("n1")
            iv = s_t("iv")
            for (sx, dx, inv) in ((sq, dq, inq), (sp, dp, inp_)):
                nc.vector.tensor_mul(t1[:], sx, sx)
                nc.vector.scalar_tensor_tensor(
                    t2[:], t1[:], -1.0 / C, dx, Alu.mult, Alu.add
                )
                nc.scalar.activation(t1[:], t2[:], Act.Sqrt)
                nc.vector.reciprocal(iv[:], t1[:])
                nc.vector.tensor_mul(n1[:], iv[:], iv[:])
                nc.vector.tensor_mul(n1[:], n1[:], t2[:])
                nc.vector.tensor_scalar(n1[:], n1[:], -0.5, 1.5, Alu.mult, Alu.add)
                nc.vector.tensor_mul(inv[:], iv[:], n1[:])
            rC = 1.0 / np.sqrt(float(C))
            nc.vector.scalar_tensor_tensor(aq[:], sq, rC, inq[:], Alu.mult, Alu.mult)
            nc.vector.scalar_tensor_tensor(ap_[:], sp, rC, inp_[:], Alu.mult, Alu.mult)

            # sim = (qtp * inq x inp) - aq x ap, in [49, 50]-padded fp16
            G3 = flatG[:].rearrange("b (q c) -> b q c", c=AC)
            qtp3 = G3[:, 0:HW, HW : 2 * HW]
            sim3 = simb[:].rearrange("b (q p) -> b q p", p=PAD)
            KS3 = KS[:].rearrange("b (q p) -> b q p", p=PAD)
            b33 = b3[:].rearrange("b (q p) -> b q p", p=PAD)
            bq = inq[:].unsqueeze(2).broadcast_to([BS, HW, HW])
            bp = inp_[:].unsqueeze(1).broadcast_to([BS, HW, HW])
            baq = aq[:].unsqueeze(2).broadcast_to([BS, HW, HW])
            bap = ap_[:].unsqueeze(1).broadcast_to([BS, HW, HW])
            nc.vector.tensor_mul(KS3[:, :, 0:HW], qtp3, bq)  # s1 (KS as scratch)
            nc.vector.tensor_mul(b33[:, :, 0:HW], baq, bap)
            nc.vector.tensor_mul(sim3[:, :, 0:HW], KS3[:, :, 0:HW], bp)  # s2
            nc.vector.tensor_sub(sim3[:, :, 0:HW], sim3[:, :, 0:HW], b33[:, :, 0:HW])

            # K = exp(sim/eps); Kt via transposed read (ACT)
            nc.scalar.activation(Kb[:], simb[:], Act.Exp, scale=EXPB)
            simT = simb[:].rearrange("b (q p) -> b p q", p=PAD)[:, 0:HW, :]
            KtV = Ktb[:].rearrange("b (p q) -> b p q", q=PAD)[:, :, 0:HW]
            nc.scalar.activation(KtV, simT, Act.Exp, scale=EXPB)
            # KS = K * sim for the final logits
            nc.vector.tensor_mul(KS[:], Kb[:], simb[:])

            # ---------------- Phase B: Sinkhorn ----------------
            K3 = Kb[:].rearrange("b (q p) -> b q p", p=PAD)
            Kt3 = Ktb[:].rearrange("b (p q) -> b p q", q=PAD)
            tb3 = tb[:].rearrange("b (x y) -> b x y", y=PAD)
            kv, rkv = s_t("kv"), s_t("rkv")
            bvs = vs[:].unsqueeze(1).broadcast_to([BS, HW, PAD])
            bus = us[:].unsqueeze(1).broadcast_to([BS, HW, PAD])
            for it in range(ITERS):
                if it == 0:
                    nc.vector.tensor_reduce(kv[:], K3, axis=AxX, op=Alu.add)
                else:
                    nc.vector.tensor_mul(tb3, K3, bvs)
                    nc.vector.tensor_reduce(kv[:], tb3, axis=AxX, op=Alu.add)
                nc.vector.reciprocal(rkv[:], kv[:])
                nc.vector.tensor_mul(us[:, 0:HW], w1f[:], rkv[:])
                if HALF and it == ITERS - 1:
                    break
                nc.vector.tensor_mul(tb3, Kt3, bus)
                nc.vector.tensor_reduce(kv[:], tb3, axis=AxX, op=Alu.add)
                nc.vector.reciprocal(rkv[:], kv[:])
                nc.vector.tensor_mul(vs[:, 0:HW], w2f[:], rkv[:])

            # ---------------- Phase B: logits ----------------
            lgr = s_t("lgr", 1)
            lgf = s_t("lgf", 1)
            nc.vector.tensor_mul(tb3, KS3, bvs)  # KS * vs
            nc.vector.tensor_reduce(kv[:], tb3, axis=AxX, op=Alu.add)
            nc.vector.tensor_mul(kv[:], kv[:], w1r[:])  # scratch use below
            nc.vector.tensor_mul(kv[:], kv[:], rkv[:])
            nc.vector.tensor_reduce(lgr[:], kv[:], axis=AxX, op=Alu.add)
            nc.vector.reciprocal(rkv[:, 0:1], s1s[:] if HALF else s2s[:])
            nc.vector.scalar_tensor_tensor(
                lgf[:], lgr[:], TEMP, rkv[:, 0:1], Alu.mult, Alu.mult
            )
            nc.sync.dma_start(outp[:, :], lgf[:])

    nc.compile()
    return nc


_NC = None


def _get_nc():
    global _NC
    if _NC is None:
        _NC = build_nc()
    return _NC


def _prep_in_maps(feature_map1, feature_map2):
    q = np.ascontiguousarray(np.asarray(feature_map1, dtype=np.float32)).reshape(
        B_FULL, C, HW
    )
    p = np.ascontiguousarray(np.asarray(feature_map2, dtype=np.float32)).reshape(
        B_FULL, C, HW
    )
    in_maps = []
    for i in range(NCORE):
        sl = slice(i * BS, (i + 1) * BS)
        a32 = np.empty((NGRP, 128, GRP, NCH, AC), np.float32)
        a32[..., AC - 1] = 1.0
        qc = q[sl].reshape(NGRP, GRP, NCH, 128, HW).transpose(0, 3, 1, 2, 4)
        pc = p[sl].reshape(NGRP, GRP, NCH, 128, HW).transpose(0, 3, 1, 2, 4)
        a32[..., 0:HW] = qc
        a32[..., HW : 2 * HW] = pc
        in_maps.append({"aug": a32.astype(np.float16).reshape(NGRP, 128, SLAB)})
    return in_maps


def run(feature_map1, feature_map2, trace=False):
    in_maps = _prep_in_maps(feature_map1, feature_map2)
    nc = _get_nc()
    res = run_bass_kernel_spmd(nc, in_maps, core_ids=list(range(NCORE)), trace=trace)
    out = np.concatenate(
        [np.asarray(res.results[i]["out"]).reshape(BS) for i in range(NCORE)]
    ).astype(np.float32)
    return out, res


def kernel(feature_map1, feature_map2):
    out, _ = run(feature_map1, feature_map2, trace=False)
    return out


# revision 28
# speedup vs baseline: 1.4364x; 1.0185x over previous
"""DeepEMD Trainium2 kernel: batched 49x49 entropic-OT (Sinkhorn) similarity.

v2 strategy (8 NeuronCores, data-parallel over batch; 128 batches/core):
- Host prepacks per (group of 32 batches) slabs A = [Q | P | 1] (128ch x 99)
  fp16, batch-major within the slab so grams start after partial loads.
- Phase A: 4 big loads (1.6MB sub-DMAs, alone on the sync queue to avoid
  head-of-line blocking), per-batch Gram G_b = A^T A via 4 accumulating
  fp16 matmuls (weights widened to 128 cols to engage fast-weight-load),
  PSUM->SBUF fp16 copies into 16-batch blocks, per-batch flatten DMAs
  (99 descs each) spread across scalar/gpsimd queues (sync joins for the
  last group). w1/w2 row/col-sum reduces run per block on the G-major
  layout during phase A; a PE transpose turns them batch-major.
- Phase B: all big tensors fp16 in a [49, 50]-padded layout so every
  tensor_tensor runs in the DVE 2x packed mode (4B-aligned step-1 rows).
  K = exp(20*sim) directly (exp((sim-1)/eps)*e^20 rescale cancels in the
  final normalization). Kt is built by ACT reading sim transposed. Linear
  Gauss-Seidel Sinkhorn, ITERS=4 with the last vs-update skipped (flow
  mass then = sum(w1), so logits normalize by 1/s1). b3/KS products run
  on gpsimd concurrently with DVE. Final logit = one fused
  tensor_tensor_reduce over the full [49,50] row.
"""

import os
import sys

import numpy as np

sys.path.insert(0, "/opt/trn_rl_repo")

import concourse.bass as bass
import concourse.bacc as bacc
import concourse.mybir as mybir
from concourse import tile
from concourse import masks
from concourse.bass_utils import run_bass_kernel_spmd

B_FULL, C, HW = 1024, 512, 49
NCORE = 8
BS = B_FULL // NCORE  # 128 batches per core
NCH = C // 128  # 4 chunks of 128 channels
AC = 2 * HW + 1  # 99 augmented cols [Q | P | 1]
GRP = 32  # batches per load group
NGRP = BS // GRP
NSPL = 2  # sub-DMAs per group load
FB = 16  # batches per gs block / flatten group
PAD = 50  # padded inner dim (4B-aligned fp16 rows)
FW = HW * PAD  # 2450
ITERS = 4
HALF = True  # skip last vs-update; normalize by s1
EXPB = 1.0 / 0.05  # K = exp(sim/eps) (the e^{-1/eps} factor cancels)
SIMPAD = -600.0
TEMP = 12.5 / HW

f32 = mybir.dt.float32
f16 = mybir.dt.float16
Alu = mybir.AluOpType
Act = mybir.ActivationFunctionType
AxX = mybir.AxisListType.X

SLAB = GRP * NCH * AC  # 12672 cols per group slab


def build_nc(debug=False):
    nc = bacc.Bacc(None, target_bir_lowering=False, debug=debug)
    aug = nc.declare_dram_parameter("aug", [NGRP, 128, SLAB], f16, isOutput=False)
    outp = nc.declare_dram_parameter("out", [BS, 1], f32, isOutput=True)

    with tile.TileContext(nc) as tc:
        with (
            tc.tile_pool(name="big", bufs=1) as big,
            tc.tile_pool(name="stage", bufs=4) as stg,
            tc.tile_pool(name="gblk", bufs=6) as gcp,
            tc.tile_pool(name="small", bufs=1) as sml,
            tc.tile_pool(name="psum", bufs=7, space="PSUM") as pp,
            tc.tile_pool(name="psumw", bufs=1, space="PSUM") as ppw,
        ):
            flatG = big.tile([BS, AC * AC], f16, tag="flatG", name="flatG")

            simb = big.tile([BS, FW], f16, tag="sim", name="sim")
            Kb = big.tile([BS, FW], f16, tag="K", name="K")
            Ktb = big.tile([BS, FW], f16, tag="Kt", name="Kt")
            tb = big.tile([BS, FW], f16, tag="tb", name="tb")
            KS = big.tile([BS, FW], f16, tag="KS", name="KS")
            b3 = big.tile([BS, FW], f16, tag="b3", name="b3")
            # pad prep: sim pads -> exp 0; Kt pad col stays 0
            nc.gpsimd.memset(simb[:], SIMPAD)
            nc.gpsimd.memset(Ktb[:], 0.0)

            def s_t(tag, w=HW, dt=f32):
                return sml.tile([BS, w], dt, tag=tag, name=tag)

            us = s_t("us", PAD, f16)
            vs = s_t("vs", PAD, f16)
            nc.gpsimd.memset(us[:], 0.0)
            nc.gpsimd.memset(vs[:], 0.0)
            # warm the ACT sqrt table early under the load shadow (Exp is
            # preloaded after the norm sqrts to avoid table thrash)
            wrm = s_t("wrm", 1)
            nc.vector.memset(wrm[:], 1.0)
            nc.scalar.activation(wrm[:], wrm[:], Act.Sqrt)

            # ---------------- Phase A: load + gram + flatten ----------------
            # All loads issue upfront (stage bufs cover all groups) so the
            # sync queue never head-of-line blocks a later load behind a
            # sem-waiting flatten.
            ths = []
            sw = SLAB // NSPL
            for g in range(NGRP):
                th = stg.tile([128, SLAB], f16, tag="h", name="hg")
                ths.append(th)
                for ss in range(NSPL):
                    nc.sync.dma_start(
                        th[:, ss * sw : (ss + 1) * sw],
                        aug[g, :, ss * sw : (ss + 1) * sw],
                    )
            gs = None
            ps = None
            CPG = 4  # grams per PSUM bank / per copy
            for g in range(NGRP):
                th = ths[g]
                for b in range(GRP):
                    gb = g * GRP + b
                    blk, bb = gb // FB, gb % FB
                    if bb == 0:
                        gs = gcp.tile([AC, FB * AC], f16, tag="gs", name="gs")
                    cb = bb % CPG
                    if cb == 0:
                        ps = pp.tile([128, CPG * AC], f32, tag="gram", name="gram")
                    # start/stop must cover one partition range: widen all 4
                    # chunks or none (no widening only for the slab tail)
                    wid = 128 if b * (NCH * AC) + (NCH - 1) * AC + 128 <= SLAB else AC
                    for j in range(NCH):
                        base = b * (NCH * AC) + j * AC
                        nc.tensor.matmul(
                            ps[0:wid, cb * AC : (cb + 1) * AC],
                            th[:, base : base + wid],
                            th[:, base : base + AC],
                            start=(j == 0),
                            stop=(j == NCH - 1),
                        )
                    if cb == CPG - 1:
                        nc.vector.tensor_copy(
                            gs[:, (bb - 3) * AC : (bb + 1) * AC], ps[0:AC, :]
                        )
                        for k in range(CPG):
                            gk = gb - (CPG - 1) + k
                            dmae = (nc.scalar, nc.gpsimd, nc.sync)[gk % 3]
                            bk = gk % FB
                            dmae.dma_start(
                                flatG[gk : gk + 1, :],
                                gs[:, bk * AC : (bk + 1) * AC],
                            )


            # ---------------- Phase B: fixups ----------------
            w1r, w2r = s_t("w1r"), s_t("w2r")
            w1f = s_t("w1f", HW, f16)
            w2f = s_t("w2f", HW, f16)
            s1s, s2s = s_t("s1s", 1), s_t("s2s", 1)
            G3e = flatG[:].rearrange("b (q c) -> b q c", c=AC)
            nc.vector.tensor_reduce(
                w1r[:], G3e[:, 0:HW, HW : 2 * HW], axis=AxX, op=Alu.add
            )
            nc.vector.tensor_reduce(
                w2r[:], G3e[:, HW : 2 * HW, 0:HW], axis=AxX, op=Alu.add
            )
            for wr, wf in ((w1r, w1f), (w2r, w2f)):
                nc.vector.tensor_scalar(wr[:], wr[:], 1.0 / HW, 0.0, Alu.mult, Alu.max)
                nc.vector.tensor_scalar(wr[:], wr[:], 0.001, None, Alu.add)
                nc.vector.tensor_copy(wf[:], wr[:])
            if HALF:
                nc.vector.tensor_reduce(s1s[:], w1r[:], axis=AxX, op=Alu.add)
            else:
                nc.vector.tensor_reduce(s2s[:], w2r[:], axis=AxX, op=Alu.add)

            # diag/sum views on flatG (fp16, stride 100 diagonals)
            def dview(row0, col0):
                v = flatG[:, row0 * AC + col0 : row0 * AC + col0 + 1].copy()
                v.ap = mybir.VecI64Pair([list(v.ap[0])] + [[AC + 1, HW]])
                return v

            dq = dview(0, 0)
            dp = dview(HW, HW)
            sq = flatG[:, (AC - 1) * AC : (AC - 1) * AC + HW]
            sp = flatG[:, (AC - 1) * AC + HW : (AC - 1) * AC + 2 * HW]

            inq = s_t("inq", HW, f16)
            inp_ = s_t("inp", HW, f16)
            aq = s_t("aq", HW, f16)
            ap_ = s_t("ap", HW, f16)
            t1, t2, n1 = s_t("t1"), s_t("t2"), s_t("n1")
            iv = s_t("iv")
            for (sx, dx, inv) in ((sq, dq, inq), (sp, dp, inp_)):
                nc.vector.tensor_mul(t1[:], sx, sx)
                nc.vector.scalar_tensor_tensor(
                    t2[:], t1[:], -1.0 / C, dx, Alu.mult, Alu.add
                )
                nc.scalar.activation(t1[:], t2[:], Act.Sqrt)
                nc.vector.reciprocal(iv[:], t1[:])
                nc.vector.tensor_mul(n1[:], iv[:], iv[:])
                nc.vector.tensor_mul(n1[:], n1[:], t2[:])
                nc.vector.tensor_scalar(n1[:], n1[:], -0.5, 1.5, Alu.mult, Alu.add)
                nc.vector.tensor_mul(inv[:], iv[:], n1[:])
            # preload the Exp table now (both Sqrt uses are done) so the
            # load hides under the sim-build DVE ops
            nc.scalar.activation(wrm[:], wrm[:], Act.Exp)
            rC = 1.0 / np.sqrt(float(C))
            nc.vector.scalar_tensor_tensor(aq[:], sq, rC, inq[:], Alu.mult, Alu.mult)
            nc.vector.scalar_tensor_tensor(ap_[:], sp, rC, inp_[:], Alu.mult, Alu.mult)

            # sim = (qtp * inq x inp) - aq x ap, in [49, 50]-padded fp16
            G3 = flatG[:].rearrange("b (q c) -> b q c", c=AC)
            qtp3 = G3[:, 0:HW, HW : 2 * HW]
            sim3 = simb[:].rearrange("b (q p) -> b q p", p=PAD)
            KS3 = KS[:].rearrange("b (q p) -> b q p", p=PAD)
            b33 = b3[:].rearrange("b (q p) -> b q p", p=PAD)
            bq = inq[:].unsqueeze(2).broadcast_to([BS, HW, HW])
            bp = inp_[:].unsqueeze(1).broadcast_to([BS, HW, HW])
            baq = aq[:].unsqueeze(2).broadcast_to([BS, HW, HW])
            bap = ap_[:].unsqueeze(1).broadcast_to([BS, HW, HW])
            nc.vector.tensor_mul(KS3[:, :, 0:HW], qtp3, bq)  # s1 (KS as scratch)
            nc.vector.tensor_mul(b33[:, :, 0:HW], baq, bap)
            nc.vector.tensor_mul(sim3[:, :, 0:HW], KS3[:, :, 0:HW], bp)  # s2
            nc.vector.tensor_sub(sim3[:, :, 0:HW], sim3[:, :, 0:HW], b33[:, :, 0:HW])

            # K = exp(sim/eps); Kt via transposed read (ACT)
            nc.scalar.activation(Kb[:], simb[:], Act.Exp, scale=EXPB)
            simT = simb[:].rearrange("b (q p) -> b p q", p=PAD)[:, 0:HW, :]
            KtV = Ktb[:].rearrange("b (p q) -> b p q", q=PAD)[:, :, 0:HW]
            nc.scalar.activation(KtV, simT, Act.Exp, scale=EXPB)
            # KS = K * sim for the final logits
            nc.vector.tensor_mul(KS[:], Kb[:], simb[:])

            # ---------------- Phase B: Sinkhorn ----------------
            K3 = Kb[:].rearrange("b (q p) -> b q p", p=PAD)
            Kt3 = Ktb[:].rearrange("b (p q) -> b p q", q=PAD)
            tb3 = tb[:].rearrange("b (x y) -> b x y", y=PAD)
            kv, rkv = s_t("kv"), s_t("rkv")
            bvs = vs[:].unsqueeze(1).broadcast_to([BS, HW, PAD])
            bus = us[:].unsqueeze(1).broadcast_to([BS, HW, PAD])
            for it in range(ITERS):
                if it == 0:
                    nc.vector.tensor_reduce(kv[:], K3, axis=AxX, op=Alu.add)
                else:
                    nc.vector.tensor_mul(tb3, K3, bvs)
                    nc.vector.tensor_reduce(kv[:], tb3, axis=AxX, op=Alu.add)
                nc.vector.reciprocal_approx_fast(rkv[:], kv[:])
                nc.vector.tensor_mul(us[:, 0:HW], w1f[:], rkv[:])
                if HALF and it == ITERS - 1:
                    break
                nc.vector.tensor_mul(tb3, Kt3, bus)
                nc.vector.tensor_reduce(kv[:], tb3, axis=AxX, op=Alu.add)
                nc.vector.reciprocal_approx_fast(rkv[:], kv[:])
                nc.vector.tensor_mul(vs[:, 0:HW], w2f[:], rkv[:])

            # ---------------- Phase B: logits ----------------
            lgr = s_t("lgr", 1)
            lgf = s_t("lgf", 1)
            nc.vector.tensor_mul(tb3, KS3, bvs)  # KS * vs
            nc.vector.tensor_reduce(kv[:], tb3, axis=AxX, op=Alu.add)
            nc.vector.tensor_mul(kv[:], kv[:], w1r[:])  # scratch use below
            nc.vector.tensor_mul(kv[:], kv[:], rkv[:])
            nc.vector.tensor_reduce(lgr[:], kv[:], axis=AxX, op=Alu.add)
            nc.vector.reciprocal(rkv[:, 0:1], s1s[:] if HALF else s2s[:])
            nc.vector.scalar_tensor_tensor(
                lgf[:], lgr[:], TEMP, rkv[:, 0:1], Alu.mult, Alu.mult
            )
            nc.sync.dma_start(outp[:, :], lgf[:])

    nc.compile()
    return nc


_NC = None


def _get_nc():
    global _NC
    if _NC is None:
        _NC = build_nc()
    return _NC


def _prep_in_maps(feature_map1, feature_map2):
    q = np.ascontiguousarray(np.asarray(feature_map1, dtype=np.float32)).reshape(
        B_FULL, C, HW
    )
    p = np.ascontiguousarray(np.asarray(feature_map2, dtype=np.float32)).reshape(
        B_FULL, C, HW
    )
    in_maps = []
    for i in range(NCORE):
        sl = slice(i * BS, (i + 1) * BS)
        a32 = np.empty((NGRP, 128, GRP, NCH, AC), np.float32)
        a32[..., AC - 1] = 1.0
        qc = q[sl].reshape(NGRP, GRP, NCH, 128, HW).transpose(0, 3, 1, 2, 4)
        pc = p[sl].reshape(NGRP, GRP, NCH, 128, HW).transpose(0, 3, 1, 2, 4)
        a32[..., 0:HW] = qc
        a32[..., HW : 2 * HW] = pc
        in_maps.append({"aug": a32.astype(np.float16).reshape(NGRP, 128, SLAB)})
    return in_maps


def run(feature_map1, feature_map2, trace=False):
    in_maps = _prep_in_maps(feature_map1, feature_map2)
    nc = _get_nc()
    res = run_bass_kernel_spmd(nc, in_maps, core_ids=list(range(NCORE)), trace=trace)
    out = np.concatenate(
        [np.asarray(res.results[i]["out"]).reshape(BS) for i in range(NCORE)]
    ).astype(np.float32)
    return out, res


def kernel(feature_map1, feature_map2):
    out, _ = run(feature_map1, feature_map2, trace=False)
    return out


# revision 31
# speedup vs baseline: 1.6672x; 1.1606x over previous
"""DeepEMD Trainium2 kernel: batched 49x49 entropic-OT (Sinkhorn) similarity.

v2 strategy (8 NeuronCores, data-parallel over batch; 128 batches/core):
- Host prepacks per (group of 32 batches) slabs A = [Q | P | 1] (128ch x 99)
  fp16, batch-major within the slab so grams start after partial loads.
- Phase A: 4 big loads (1.6MB sub-DMAs, alone on the sync queue to avoid
  head-of-line blocking), per-batch Gram G_b = A^T A via 4 accumulating
  fp16 matmuls (weights widened to 128 cols to engage fast-weight-load),
  PSUM->SBUF fp16 copies into 16-batch blocks, per-batch flatten DMAs
  (99 descs each) spread across scalar/gpsimd queues (sync joins for the
  last group). w1/w2 row/col-sum reduces run per block on the G-major
  layout during phase A; a PE transpose turns them batch-major.
- Phase B: all big tensors fp16 in a [49, 50]-padded layout so every
  tensor_tensor runs in the DVE 2x packed mode (4B-aligned step-1 rows).
  K = exp(20*sim) directly (exp((sim-1)/eps)*e^20 rescale cancels in the
  final normalization). Kt is built by ACT reading sim transposed. Linear
  Gauss-Seidel Sinkhorn, ITERS=4 with the last vs-update skipped (flow
  mass then = sum(w1), so logits normalize by 1/s1). b3/KS products run
  on gpsimd concurrently with DVE. Final logit = one fused
  tensor_tensor_reduce over the full [49,50] row.
"""

import os
import sys

import numpy as np

sys.path.insert(0, "/opt/trn_rl_repo")

import concourse.bass as bass
import concourse.bacc as bacc
import concourse.mybir as mybir
from concourse import tile
from concourse import masks
from concourse.bass_utils import run_bass_kernel_spmd

B_FULL, C, HW = 1024, 512, 49
NCORE = 8
BS = B_FULL // NCORE  # 128 batches per core
NCH = C // 128  # 4 chunks of 128 channels
AC = 2 * HW + 1  # 99 augmented cols [Q | P | 1]
GRP = 32  # batches per load group
NGRP = BS // GRP
NSPL = 1  # sub-DMAs per group load
FB = 16  # batches per gs block / flatten group
PAD = 50  # padded inner dim (4B-aligned fp16 rows)
FW = HW * PAD  # 2450
ITERS = 4
HALF = True  # skip last vs-update; normalize by s1
EXPB = 1.0 / 0.05  # K = exp(sim/eps) (the e^{-1/eps} factor cancels)
SIMPAD = -600.0
TEMP = 12.5 / HW

f32 = mybir.dt.float32
f16 = mybir.dt.float16
Alu = mybir.AluOpType
Act = mybir.ActivationFunctionType
AxX = mybir.AxisListType.X

SLAB = GRP * NCH * AC  # 12672 cols per group slab


def build_nc(debug=False):
    nc = bacc.Bacc(None, target_bir_lowering=False, debug=debug)
    aug = nc.declare_dram_parameter("aug", [NGRP, 128, SLAB], f16, isOutput=False)
    outp = nc.declare_dram_parameter("out", [BS, 1], f32, isOutput=True)

    with tile.TileContext(nc) as tc:
        with (
            tc.tile_pool(name="big", bufs=1) as big,
            tc.tile_pool(name="stage", bufs=4) as stg,
            tc.tile_pool(name="gblk", bufs=8) as gcp,
            tc.tile_pool(name="small", bufs=1) as sml,
            tc.tile_pool(name="psum", bufs=7, space="PSUM") as pp,
            tc.tile_pool(name="psumw", bufs=1, space="PSUM") as ppw,
        ):
            flatG = big.tile([BS, AC * AC], f16, tag="flatG", name="flatG")

            simb = big.tile([BS, FW], f16, tag="sim", name="sim")
            Kb = big.tile([BS, FW], f16, tag="K", name="K")
            Ktb = big.tile([BS, FW], f16, tag="Kt", name="Kt")
            tb = big.tile([BS, FW], f16, tag="tb", name="tb")
            KS = big.tile([BS, FW], f16, tag="KS", name="KS")
            b3 = big.tile([BS, FW], f16, tag="b3", name="b3")
            # pad prep: sim pads -> exp 0; Kt pad col stays 0
            nc.gpsimd.memset(simb[:], SIMPAD)
            nc.gpsimd.memset(Ktb[:], 0.0)

            def s_t(tag, w=HW, dt=f32):
                return sml.tile([BS, w], dt, tag=tag, name=tag)

            us = s_t("us", PAD, f16)
            vs = s_t("vs", PAD, f16)
            nc.gpsimd.memset(us[:], 0.0)
            nc.gpsimd.memset(vs[:], 0.0)
            # warm the ACT sqrt table early under the load shadow (Exp is
            # preloaded after the norm sqrts to avoid table thrash)
            wrm = s_t("wrm", 1)
            nc.vector.memset(wrm[:], 1.0)
            nc.scalar.activation(wrm[:], wrm[:], Act.Sqrt)

            # ---------------- Phase A: load + gram + flatten ----------------
            # All loads issue upfront (stage bufs cover all groups) so the
            # sync queue never head-of-line blocks a later load behind a
            # sem-waiting flatten.
            ths = []
            sw = SLAB // NSPL
            for g in range(NGRP):
                th = stg.tile([128, SLAB], f16, tag="h", name="hg")
                ths.append(th)
                for ss in range(NSPL):
                    nc.sync.dma_start(
                        th[:, ss * sw : (ss + 1) * sw],
                        aug[g, :, ss * sw : (ss + 1) * sw],
                    )
            gs = None
            ps = None
            CPG = 4  # grams per PSUM bank / per copy
            for g in range(NGRP):
                th = ths[g]
                for b in range(GRP):
                    gb = g * GRP + b
                    blk, bb = gb // FB, gb % FB
                    if bb == 0:
                        gs = gcp.tile([AC, FB * AC], f16, tag="gs", name="gs")
                    cb = bb % CPG
                    if cb == 0:
                        ps = pp.tile([128, CPG * AC], f32, tag="gram", name="gram")
                    # start/stop must cover one partition range: widen all 4
                    # chunks or none (no widening only for the slab tail)
                    wid = 128 if b * (NCH * AC) + (NCH - 1) * AC + 128 <= SLAB else AC
                    for j in range(NCH):
                        base = b * (NCH * AC) + j * AC
                        nc.tensor.matmul(
                            ps[0:wid, cb * AC : (cb + 1) * AC],
                            th[:, base : base + wid],
                            th[:, base : base + AC],
                            start=(j == 0),
                            stop=(j == NCH - 1),
                        )
                    if cb == CPG - 1:
                        nc.vector.tensor_copy(
                            gs[:, (bb - 3) * AC : (bb + 1) * AC], ps[0:AC, :]
                        )
                        # SWDGE drains flattens ~1.6x faster than the HWDGE
                        # rings; weight the rotation accordingly
                        ROT = (
                            nc.gpsimd, nc.scalar, nc.sync,
                            nc.gpsimd, nc.scalar, nc.gpsimd, nc.sync,
                        )
                        for k in range(CPG):
                            gk = gb - (CPG - 1) + k
                            dmae = ROT[gk % len(ROT)]
                            bk = gk % FB
                            dmae.dma_start(
                                flatG[gk : gk + 1, :],
                                gs[:, bk * AC : (bk + 1) * AC],
                            )


            # ---------------- Phase B: fixups ----------------
            w1r, w2r = s_t("w1r"), s_t("w2r")
            w1f = s_t("w1f", HW, f16)
            w2f = s_t("w2f", HW, f16)
            s1s, s2s = s_t("s1s", 1), s_t("s2s", 1)
            G3e = flatG[:].rearrange("b (q c) -> b q c", c=AC)
            nc.vector.tensor_reduce(
                w1r[:], G3e[:, 0:HW, HW : 2 * HW], axis=AxX, op=Alu.add
            )
            nc.vector.tensor_reduce(
                w2r[:], G3e[:, HW : 2 * HW, 0:HW], axis=AxX, op=Alu.add
            )
            for wr, wf in ((w1r, w1f), (w2r, w2f)):
                nc.vector.tensor_scalar(wr[:], wr[:], 1.0 / HW, 0.0, Alu.mult, Alu.max)
                nc.vector.tensor_scalar(wr[:], wr[:], 0.001, None, Alu.add)
                nc.vector.tensor_copy(wf[:], wr[:])
            if HALF:
                nc.vector.tensor_reduce(s1s[:], w1r[:], axis=AxX, op=Alu.add)
            else:
                nc.vector.tensor_reduce(s2s[:], w2r[:], axis=AxX, op=Alu.add)

            # diag/sum views on flatG (fp16, stride 100 diagonals)
            def dview(row0, col0):
                v = flatG[:, row0 * AC + col0 : row0 * AC + col0 + 1].copy()
                v.ap = mybir.VecI64Pair([list(v.ap[0])] + [[AC + 1, HW]])
                return v

            dq = dview(0, 0)
            dp = dview(HW, HW)
            sq = flatG[:, (AC - 1) * AC : (AC - 1) * AC + HW]
            sp = flatG[:, (AC - 1) * AC + HW : (AC - 1) * AC + 2 * HW]

            inq = s_t("inq", HW, f16)
            inp_ = s_t("inp", HW, f16)
            aq = s_t("aq", HW, f16)
            ap_ = s_t("ap", HW, f16)
            t1, t2, n1 = s_t("t1"), s_t("t2"), s_t("n1")
            iv = s_t("iv")
            for (sx, dx, inv) in ((sq, dq, inq), (sp, dp, inp_)):
                nc.vector.tensor_mul(t1[:], sx, sx)
                nc.vector.scalar_tensor_tensor(
                    t2[:], t1[:], -1.0 / C, dx, Alu.mult, Alu.add
                )
                nc.scalar.activation(t1[:], t2[:], Act.Sqrt)
                nc.vector.reciprocal(iv[:], t1[:])
                nc.vector.tensor_mul(n1[:], iv[:], iv[:])
                nc.vector.tensor_mul(n1[:], n1[:], t2[:])
                nc.vector.tensor_scalar(n1[:], n1[:], -0.5, 1.5, Alu.mult, Alu.add)
                nc.vector.tensor_mul(inv[:], iv[:], n1[:])
            # preload the Exp table now (both Sqrt uses are done) so the
            # load hides under the sim-build DVE ops
            nc.scalar.activation(wrm[:], wrm[:], Act.Exp)
            rC = 1.0 / np.sqrt(float(C))
            nc.vector.scalar_tensor_tensor(aq[:], sq, rC, inq[:], Alu.mult, Alu.mult)
            nc.vector.scalar_tensor_tensor(ap_[:], sp, rC, inp_[:], Alu.mult, Alu.mult)

            # sim = (qtp * inq x inp) - aq x ap, in [49, 50]-padded fp16
            G3 = flatG[:].rearrange("b (q c) -> b q c", c=AC)
            qtp3 = G3[:, 0:HW, HW : 2 * HW]
            sim3 = simb[:].rearrange("b (q p) -> b q p", p=PAD)
            KS3 = KS[:].rearrange("b (q p) -> b q p", p=PAD)
            b33 = b3[:].rearrange("b (q p) -> b q p", p=PAD)
            bq = inq[:].unsqueeze(2).broadcast_to([BS, HW, HW])
            bp = inp_[:].unsqueeze(1).broadcast_to([BS, HW, HW])
            baq = aq[:].unsqueeze(2).broadcast_to([BS, HW, HW])
            bap = ap_[:].unsqueeze(1).broadcast_to([BS, HW, HW])
            nc.vector.tensor_mul(KS3[:, :, 0:HW], qtp3, bq)  # s1 (KS as scratch)
            nc.vector.tensor_mul(b33[:, :, 0:HW], baq, bap)
            nc.vector.tensor_mul(sim3[:, :, 0:HW], KS3[:, :, 0:HW], bp)  # s2
            nc.vector.tensor_sub(sim3[:, :, 0:HW], sim3[:, :, 0:HW], b33[:, :, 0:HW])

            # K = exp(sim/eps); Kt via transposed read (ACT)
            nc.scalar.activation(Kb[:], simb[:], Act.Exp, scale=EXPB)
            simT = simb[:].rearrange("b (q p) -> b p q", p=PAD)[:, 0:HW, :]
            KtV = Ktb[:].rearrange("b (p q) -> b p q", q=PAD)[:, :, 0:HW]
            nc.scalar.activation(KtV, simT, Act.Exp, scale=EXPB)
            # KS = K * sim for the final logits
            nc.vector.tensor_mul(KS[:], Kb[:], simb[:])

            # ---------------- Phase B: Sinkhorn ----------------
            K3 = Kb[:].rearrange("b (q p) -> b q p", p=PAD)
            Kt3 = Ktb[:].rearrange("b (p q) -> b p q", q=PAD)
            tb3 = tb[:].rearrange("b (x y) -> b x y", y=PAD)
            kv, rkv = s_t("kv"), s_t("rkv")
            bvs = vs[:].unsqueeze(1).broadcast_to([BS, HW, PAD])
            bus = us[:].unsqueeze(1).broadcast_to([BS, HW, PAD])
            for it in range(ITERS):
                if it == 0:
                    nc.vector.tensor_reduce(kv[:], K3, axis=AxX, op=Alu.add)
                else:
                    nc.vector.tensor_mul(tb3, K3, bvs)
                    nc.vector.tensor_reduce(kv[:], tb3, axis=AxX, op=Alu.add)
                nc.vector.reciprocal_approx_fast(rkv[:], kv[:])
                nc.vector.tensor_mul(us[:, 0:HW], w1f[:], rkv[:])
                if HALF and it == ITERS - 1:
                    break
                nc.vector.tensor_mul(tb3, Kt3, bus)
                nc.vector.tensor_reduce(kv[:], tb3, axis=AxX, op=Alu.add)
                nc.vector.reciprocal_approx_fast(rkv[:], kv[:])
                nc.vector.tensor_mul(vs[:, 0:HW], w2f[:], rkv[:])

            # ---------------- Phase B: logits ----------------
            lgr = s_t("lgr", 1)
            lgf = s_t("lgf", 1)
            nc.vector.tensor_mul(tb3, KS3, bvs)  # KS * vs
            nc.vector.tensor_reduce(kv[:], tb3, axis=AxX, op=Alu.add)
            nc.vector.tensor_mul(kv[:], kv[:], w1r[:])  # scratch use below
            nc.vector.tensor_mul(kv[:], kv[:], rkv[:])
            nc.vector.tensor_reduce(lgr[:], kv[:], axis=AxX, op=Alu.add)
            nc.vector.reciprocal(rkv[:, 0:1], s1s[:] if HALF else s2s[:])
            nc.vector.scalar_tensor_tensor(
                lgf[:], lgr[:], TEMP, rkv[:, 0:1], Alu.mult, Alu.mult
            )
            nc.sync.dma_start(outp[:, :], lgf[:])

    nc.compile()
    return nc


_NC = None


def _get_nc():
    global _NC
    if _NC is None:
        _NC = build_nc()
    return _NC


def _prep_in_maps(feature_map1, feature_map2):
    q = np.ascontiguousarray(np.asarray(feature_map1, dtype=np.float32)).reshape(
        B_FULL, C, HW
    )
    p = np.ascontiguousarray(np.asarray(feature_map2, dtype=np.float32)).reshape(
        B_FULL, C, HW
    )
    in_maps = []
    for i in range(NCORE):
        sl = slice(i * BS, (i + 1) * BS)
        a32 = np.empty((NGRP, 128, GRP, NCH, AC), np.float32)
        a32[..., AC - 1] = 1.0
        qc = q[sl].reshape(NGRP, GRP, NCH, 128, HW).transpose(0, 3, 1, 2, 4)
        pc = p[sl].reshape(NGRP, GRP, NCH, 128, HW).transpose(0, 3, 1, 2, 4)
        a32[..., 0:HW] = qc
        a32[..., HW : 2 * HW] = pc
        in_maps.append({"aug": a32.astype(np.float16).reshape(NGRP, 128, SLAB)})
    return in_maps


def run(feature_map1, feature_map2, trace=False):
    in_maps = _prep_in_maps(feature_map1, feature_map2)
    nc = _get_nc()
    res = run_bass_kernel_spmd(nc, in_maps, core_ids=list(range(NCORE)), trace=trace)
    out = np.concatenate(
        [np.asarray(res.results[i]["out"]).reshape(BS) for i in range(NCORE)]
    ).astype(np.float32)
    return out, res


def kernel(feature_map1, feature_map2):
    out, _ = run(feature_map1, feature_map2, trace=False)
    return out


# revision 32
# speedup vs baseline: 1.7258x; 1.0351x over previous
"""DeepEMD Trainium2 kernel: batched 49x49 entropic-OT (Sinkhorn) similarity.

v2 strategy (8 NeuronCores, data-parallel over batch; 128 batches/core):
- Host prepacks per (group of 32 batches) slabs A = [Q | P | 1] (128ch x 99)
  fp16, batch-major within the slab so grams start after partial loads.
- Phase A: 4 big loads (1.6MB sub-DMAs, alone on the sync queue to avoid
  head-of-line blocking), per-batch Gram G_b = A^T A via 4 accumulating
  fp16 matmuls (weights widened to 128 cols to engage fast-weight-load),
  PSUM->SBUF fp16 copies into 16-batch blocks, per-batch flatten DMAs
  (99 descs each) spread across scalar/gpsimd queues (sync joins for the
  last group). w1/w2 row/col-sum reduces run per block on the G-major
  layout during phase A; a PE transpose turns them batch-major.
- Phase B: all big tensors fp16 in a [49, 50]-padded layout so every
  tensor_tensor runs in the DVE 2x packed mode (4B-aligned step-1 rows).
  K = exp(20*sim) directly (exp((sim-1)/eps)*e^20 rescale cancels in the
  final normalization). Kt is built by ACT reading sim transposed. Linear
  Gauss-Seidel Sinkhorn, ITERS=4 with the last vs-update skipped (flow
  mass then = sum(w1), so logits normalize by 1/s1). b3/KS products run
  on gpsimd concurrently with DVE. Final logit = one fused
  tensor_tensor_reduce over the full [49,50] row.
"""

import os
import sys

import numpy as np

sys.path.insert(0, "/opt/trn_rl_repo")

import concourse.bass as bass
import concourse.bacc as bacc
import concourse.mybir as mybir
from concourse import tile
from concourse import masks
from concourse.bass_utils import run_bass_kernel_spmd

B_FULL, C, HW = 1024, 512, 49
NCORE = 8
BS = B_FULL // NCORE  # 128 batches per core
NCH = C // 128  # 4 chunks of 128 channels
AC = 2 * HW + 1  # 99 augmented cols [Q | P | 1]
GRP = 32  # batches per load group
NGRP = BS // GRP
NSPL = 1  # sub-DMAs per group load
FB = 16  # batches per gs block / flatten group
PAD = 50  # padded inner dim (4B-aligned fp16 rows)
FW = HW * PAD  # 2450
ITERS = 4
HALF = True  # skip last vs-update; normalize by s1
EXPB = 1.0 / 0.05  # K = exp(sim/eps) (the e^{-1/eps} factor cancels)
SIMPAD = -600.0
TEMP = 12.5 / HW

f32 = mybir.dt.float32
f16 = mybir.dt.float16
Alu = mybir.AluOpType
Act = mybir.ActivationFunctionType
AxX = mybir.AxisListType.X

SLAB = GRP * NCH * AC  # 12672 cols per group slab


def build_nc(debug=False):
    nc = bacc.Bacc(None, target_bir_lowering=False, debug=debug)
    aug = nc.declare_dram_parameter("aug", [NGRP, 128, SLAB], f16, isOutput=False)
    outp = nc.declare_dram_parameter("out", [BS, 1], f32, isOutput=True)

    with tile.TileContext(nc) as tc:
        with (
            tc.tile_pool(name="big", bufs=1) as big,
            tc.tile_pool(name="stage", bufs=4) as stg,
            tc.tile_pool(name="gblk", bufs=8) as gcp,
            tc.tile_pool(name="small", bufs=1) as sml,
            tc.tile_pool(name="psum", bufs=7, space="PSUM") as pp,
            tc.tile_pool(name="psumw", bufs=1, space="PSUM") as ppw,
        ):
            flatG = big.tile([BS, AC * AC], f16, tag="flatG", name="flatG")

            simb = big.tile([BS, FW], f16, tag="sim", name="sim")
            Kb = big.tile([BS, FW], f16, tag="K", name="K")
            Ktb = big.tile([BS, FW], f16, tag="Kt", name="Kt")
            tb = big.tile([BS, FW], f16, tag="tb", name="tb")
            KS = big.tile([BS, FW], f16, tag="KS", name="KS")
            b3 = big.tile([BS, FW], f16, tag="b3", name="b3")
            # pad prep: sim pads -> exp 0; Kt pad col stays 0
            nc.gpsimd.memset(simb[:], SIMPAD)
            nc.gpsimd.memset(Ktb[:], 0.0)

            def s_t(tag, w=HW, dt=f32):
                return sml.tile([BS, w], dt, tag=tag, name=tag)

            us = s_t("us", PAD, f16)
            vs = s_t("vs", PAD, f16)
            nc.gpsimd.memset(us[:], 0.0)
            nc.gpsimd.memset(vs[:], 0.0)
            # warm the ACT sqrt table early under the load shadow (Exp is
            # preloaded after the norm sqrts to avoid table thrash)
            wrm = s_t("wrm", 1)
            nc.vector.memset(wrm[:], 1.0)
            nc.scalar.activation(wrm[:], wrm[:], Act.Sqrt)

            # ---------------- Phase A: load + gram + flatten ----------------
            # All loads issue upfront (stage bufs cover all groups) so the
            # sync queue never head-of-line blocks a later load behind a
            # sem-waiting flatten.
            ths = []
            sw = SLAB // NSPL
            for g in range(NGRP):
                th = stg.tile([128, SLAB], f16, tag="h", name="hg")
                ths.append(th)
                for ss in range(NSPL):
                    nc.sync.dma_start(
                        th[:, ss * sw : (ss + 1) * sw],
                        aug[g, :, ss * sw : (ss + 1) * sw],
                    )
            gs = None
            ps = None
            CPG = 4  # grams per PSUM bank / per copy
            for g in range(NGRP):
                th = ths[g]
                for b in range(GRP):
                    gb = g * GRP + b
                    blk, bb = gb // FB, gb % FB
                    if bb == 0:
                        gs = gcp.tile([AC, FB * AC], f16, tag="gs", name="gs")
                    cb = bb % CPG
                    if cb == 0:
                        ps = pp.tile([128, CPG * AC], f32, tag="gram", name="gram")
                    # start/stop must cover one partition range: widen all 4
                    # chunks or none (no widening only for the slab tail)
                    wid = 128 if b * (NCH * AC) + (NCH - 1) * AC + 128 <= SLAB else AC
                    for j in range(NCH):
                        base = b * (NCH * AC) + j * AC
                        nc.tensor.matmul(
                            ps[0:wid, cb * AC : (cb + 1) * AC],
                            th[:, base : base + wid],
                            th[:, base : base + AC],
                            start=(j == 0),
                            stop=(j == NCH - 1),
                        )
                    if cb == CPG - 1:
                        nc.vector.tensor_copy(
                            gs[:, (bb - 3) * AC : (bb + 1) * AC], ps[0:AC, :]
                        )
                        # SWDGE drains flattens ~1.74x faster than the HWDGE
                        # rings; weight the rotation accordingly (7:4:4)
                        ROT = (
                            nc.gpsimd, nc.scalar, nc.sync, nc.gpsimd,
                            nc.gpsimd, nc.scalar, nc.sync, nc.gpsimd,
                            nc.gpsimd, nc.scalar, nc.sync, nc.gpsimd,
                            nc.gpsimd, nc.scalar, nc.sync,
                        )
                        for k in range(CPG):
                            gk = gb - (CPG - 1) + k
                            dmae = ROT[gk % len(ROT)]
                            bk = gk % FB
                            dmae.dma_start(
                                flatG[gk : gk + 1, :],
                                gs[:, bk * AC : (bk + 1) * AC],
                            )


            # ---------------- Phase B: fixups ----------------
            w1r, w2r = s_t("w1r"), s_t("w2r")
            w1f = s_t("w1f", HW, f16)
            w2f = s_t("w2f", HW, f16)
            s1s, s2s = s_t("s1s", 1), s_t("s2s", 1)
            G3e = flatG[:].rearrange("b (q c) -> b q c", c=AC)
            nc.vector.tensor_reduce(
                w1r[:], G3e[:, 0:HW, HW : 2 * HW], axis=AxX, op=Alu.add
            )
            nc.vector.tensor_reduce(
                w2r[:], G3e[:, HW : 2 * HW, 0:HW], axis=AxX, op=Alu.add
            )
            for wr, wf in ((w1r, w1f), (w2r, w2f)):
                nc.vector.tensor_scalar(wr[:], wr[:], 1.0 / HW, 0.0, Alu.mult, Alu.max)
                nc.vector.tensor_scalar(wr[:], wr[:], 0.001, None, Alu.add)
                nc.vector.tensor_copy(wf[:], wr[:])
            if HALF:
                nc.vector.tensor_reduce(s1s[:], w1r[:], axis=AxX, op=Alu.add)
            else:
                nc.vector.tensor_reduce(s2s[:], w2r[:], axis=AxX, op=Alu.add)

            # diag/sum views on flatG (fp16, stride 100 diagonals)
            def dview(row0, col0):
                v = flatG[:, row0 * AC + col0 : row0 * AC + col0 + 1].copy()
                v.ap = mybir.VecI64Pair([list(v.ap[0])] + [[AC + 1, HW]])
                return v

            dq = dview(0, 0)
            dp = dview(HW, HW)
            sq = flatG[:, (AC - 1) * AC : (AC - 1) * AC + HW]
            sp = flatG[:, (AC - 1) * AC + HW : (AC - 1) * AC + 2 * HW]

            inq = s_t("inq", HW, f16)
            inp_ = s_t("inp", HW, f16)
            aq = s_t("aq", HW, f16)
            ap_ = s_t("ap", HW, f16)
            t1, t2, n1 = s_t("t1"), s_t("t2"), s_t("n1")
            iv = s_t("iv")
            for (sx, dx, inv) in ((sq, dq, inq), (sp, dp, inp_)):
                nc.vector.tensor_mul(t1[:], sx, sx)
                nc.vector.scalar_tensor_tensor(
                    t2[:], t1[:], -1.0 / C, dx, Alu.mult, Alu.add
                )
                nc.scalar.activation(t1[:], t2[:], Act.Sqrt)
                nc.vector.reciprocal(iv[:], t1[:])
                nc.vector.tensor_mul(n1[:], iv[:], iv[:])
                nc.vector.tensor_mul(n1[:], n1[:], t2[:])
                nc.vector.tensor_scalar(n1[:], n1[:], -0.5, 1.5, Alu.mult, Alu.add)
                nc.vector.tensor_mul(inv[:], iv[:], n1[:])
            # preload the Exp table now (both Sqrt uses are done) so the
            # load hides under the sim-build DVE ops
            nc.scalar.activation(wrm[:], wrm[:], Act.Exp)
            rC = 1.0 / np.sqrt(float(C))
            nc.vector.scalar_tensor_tensor(aq[:], sq, rC, inq[:], Alu.mult, Alu.mult)
            nc.vector.scalar_tensor_tensor(ap_[:], sp, rC, inp_[:], Alu.mult, Alu.mult)

            # sim = (qtp * inq x inp) - aq x ap, in [49, 50]-padded fp16
            G3 = flatG[:].rearrange("b (q c) -> b q c", c=AC)
            qtp3 = G3[:, 0:HW, HW : 2 * HW]
            sim3 = simb[:].rearrange("b (q p) -> b q p", p=PAD)
            KS3 = KS[:].rearrange("b (q p) -> b q p", p=PAD)
            b33 = b3[:].rearrange("b (q p) -> b q p", p=PAD)
            bq = inq[:].unsqueeze(2).broadcast_to([BS, HW, HW])
            bp = inp_[:].unsqueeze(1).broadcast_to([BS, HW, HW])
            baq = aq[:].unsqueeze(2).broadcast_to([BS, HW, HW])
            bap = ap_[:].unsqueeze(1).broadcast_to([BS, HW, HW])
            nc.vector.tensor_mul(KS3[:, :, 0:HW], qtp3, bq)  # s1 (KS as scratch)
            nc.vector.tensor_mul(b33[:, :, 0:HW], baq, bap)
            nc.vector.tensor_mul(sim3[:, :, 0:HW], KS3[:, :, 0:HW], bp)  # s2
            nc.vector.tensor_sub(sim3[:, :, 0:HW], sim3[:, :, 0:HW], b33[:, :, 0:HW])

            # K = exp(sim/eps); Kt via transposed read (ACT)
            nc.scalar.activation(Kb[:], simb[:], Act.Exp, scale=EXPB)
            simT = simb[:].rearrange("b (q p) -> b p q", p=PAD)[:, 0:HW, :]
            KtV = Ktb[:].rearrange("b (p q) -> b p q", q=PAD)[:, :, 0:HW]
            nc.scalar.activation(KtV, simT, Act.Exp, scale=EXPB)
            # KS = K * sim for the final logits
            nc.vector.tensor_mul(KS[:], Kb[:], simb[:])

            # ---------------- Phase B: Sinkhorn ----------------
            K3 = Kb[:].rearrange("b (q p) -> b q p", p=PAD)
            Kt3 = Ktb[:].rearrange("b (p q) -> b p q", q=PAD)
            tb3 = tb[:].rearrange("b (x y) -> b x y", y=PAD)
            kv, rkv = s_t("kv"), s_t("rkv")
            bvs = vs[:].unsqueeze(1).broadcast_to([BS, HW, PAD])
            bus = us[:].unsqueeze(1).broadcast_to([BS, HW, PAD])
            for it in range(ITERS):
                if it == 0:
                    nc.vector.tensor_reduce(kv[:], K3, axis=AxX, op=Alu.add)
                else:
                    nc.vector.tensor_mul(tb3, K3, bvs)
                    nc.vector.tensor_reduce(kv[:], tb3, axis=AxX, op=Alu.add)
                nc.vector.reciprocal_approx_fast(rkv[:], kv[:])
                nc.vector.tensor_mul(us[:, 0:HW], w1f[:], rkv[:])
                if HALF and it == ITERS - 1:
                    break
                nc.vector.tensor_mul(tb3, Kt3, bus)
                nc.vector.tensor_reduce(kv[:], tb3, axis=AxX, op=Alu.add)
                nc.vector.reciprocal_approx_fast(rkv[:], kv[:])
                nc.vector.tensor_mul(vs[:, 0:HW], w2f[:], rkv[:])

            # ---------------- Phase B: logits ----------------
            lgr = s_t("lgr", 1)
            lgf = s_t("lgf", 1)
            nc.vector.tensor_mul(tb3, KS3, bvs)  # KS * vs
            nc.vector.tensor_reduce(kv[:], tb3, axis=AxX, op=Alu.add)
            nc.vector.tensor_mul(kv[:], kv[:], w1r[:])  # scratch use below
            nc.vector.tensor_mul(kv[:], kv[:], rkv[:])
            nc.vector.tensor_reduce(lgr[:], kv[:], axis=AxX, op=Alu.add)
            nc.vector.reciprocal(rkv[:, 0:1], s1s[:] if HALF else s2s[:])
            nc.vector.scalar_tensor_tensor(
                lgf[:], lgr[:], TEMP, rkv[:, 0:1], Alu.mult, Alu.mult
            )
            nc.sync.dma_start(outp[:, :], lgf[:])

    nc.compile()
    return nc


_NC = None


def _get_nc():
    global _NC
    if _NC is None:
        _NC = build_nc()
    return _NC


def _prep_in_maps(feature_map1, feature_map2):
    q = np.ascontiguousarray(np.asarray(feature_map1, dtype=np.float32)).reshape(
        B_FULL, C, HW
    )
    p = np.ascontiguousarray(np.asarray(feature_map2, dtype=np.float32)).reshape(
        B_FULL, C, HW
    )
    in_maps = []
    for i in range(NCORE):
        sl = slice(i * BS, (i + 1) * BS)
        a32 = np.empty((NGRP, 128, GRP, NCH, AC), np.float32)
        a32[..., AC - 1] = 1.0
        qc = q[sl].reshape(NGRP, GRP, NCH, 128, HW).transpose(0, 3, 1, 2, 4)
        pc = p[sl].reshape(NGRP, GRP, NCH, 128, HW).transpose(0, 3, 1, 2, 4)
        a32[..., 0:HW] = qc
        a32[..., HW : 2 * HW] = pc
        in_maps.append({"aug": a32.astype(np.float16).reshape(NGRP, 128, SLAB)})
    return in_maps


def run(feature_map1, feature_map2, trace=False):
    in_maps = _prep_in_maps(feature_map1, feature_map2)
    nc = _get_nc()
    res = run_bass_kernel_spmd(nc, in_maps, core_ids=list(range(NCORE)), trace=trace)
    out = np.concatenate(
        [np.asarray(res.results[i]["out"]).reshape(BS) for i in range(NCORE)]
    ).astype(np.float32)
    return out, res


def kernel(feature_map1, feature_map2):
    out, _ = run(feature_map1, feature_map2, trace=False)
    return out
